# revision 6
# baseline (speedup 1.0000x reference)
"""Trainium2 Bass kernel for combined cross-entropy + batch-hard triplet loss.

Problem (N=4096, C=751, D=2048, 1024 identities x 4 instances):
  loss = mean(-log_softmax(logits)[i, t_i]) +
         mean(relu(max_same(dist) - min_diff(dist) + 0.5))
  with dist = pairwise Euclidean distances of feat rows.

v2 design (row-parallel over 8 cores, 512 rows each):
- feat is quantized to fp8e4m3 on the host; the Gram matrix block
  [512, 4096] is computed with DoubleRow fp8 matmuls (2 K-chunks of 128
  per instruction, 2x PE rate). sq_i is recomputed from the QUANTIZED
  features so d2 = sq_i + sq_j - 2*G is exactly the distance matrix of
  the quantized features (error vs fp32 reference ~2e-2 absolute on
  d~64, far inside the 2e-2 relative tolerance on the scalar loss).
- Stationary-weight reuse: for each (m-tile, k-pair) the weight load is
  shared by all 8 column blocks (psum banks 0..7 hold one full m-row of
  the Gram block), cutting LDWEIGHTS traffic 8x vs one-load-per-matmul.
- A K=34 bf16 "fold" matmul adds -sq_j/2 (split hi/lo for exactness)
  and -65536 on same-identity pairs (rows pre-sorted by target on the
  host -> block-diagonal 4-row groups; the mask data is per-core, the
  program is uniform across cores).
- Mining on device: row max of psum over all 4096 cols -> hardest
  negative; row min over the m-tile's own 128-col diagonal window ->
  hardest positive (the -65536 mask guarantees the masked entries win
  the min). The sqrt/relu/margin tail runs on the host (4096 rows).
- Cross entropy: logits in bf16, device computes row max and
  sum(exp(l - max)) via one ACT Exp with fused accumulation; host does
  ln, adds the target logit (host gather) and averages.

Per-core output [128, 16]: cols 0..3 row-max(psum) per m-tile, 4..7
row-min(window) per m-tile, 8..11 logits row max, 12..15 exp sums.
"""

import os
import sys

if "/opt/trn_rl_repo" not in sys.path:
    sys.path.insert(0, "/opt/trn_rl_repo")

import numpy as np
import ml_dtypes

N = 4096
D = 2048
C = 751
NCORES = 8
RPC = N // NCORES          # rows per core = 512
MT = RPC // 128            # 128-row tiles per core = 4
NB = N // 512              # 512-wide column blocks = 8
KT = D // 128              # 128-row contraction chunks = 16
KF = 34                    # fold contraction: 2 sq rows + 32 mask rows
BIG = 131072.0             # 2^17 offset on same pairs in q = -2*psum
MASK_SCALE = 256.0         # 2^8, exact in bf16/fp8
ALPHA = 1.0
BETA = 1.0
MARGIN = 0.5

GRAM_MODE = os.environ.get("GRAM_MODE", "fp8")   # "fp8" | "bf16"

_compiled = {}


def _build_nc():
    import concourse.bass as bass  # noqa: F401
    import concourse.tile as tile
    from concourse import mybir, bacc
    from contextlib import ExitStack

    f32 = mybir.dt.float32
    bf16 = mybir.dt.bfloat16
    f8 = mybir.dt.float8e4
    gdt = f8 if GRAM_MODE == "fp8" else bf16
    Alu = mybir.AluOpType
    Act = mybir.ActivationFunctionType
    X = mybir.AxisListType.X
    DR = mybir.MatmulPerfMode.DoubleRow if GRAM_MODE == "fp8" else None

    nc = bacc.Bacc("TRN2", target_bir_lowering=False, debug=False)

    fTq_in = nc.dram_tensor("fTq", [D, N], gdt, kind="ExternalInput").ap()
    lhq_in = nc.dram_tensor("lhq", [D, RPC], gdt, kind="ExternalInput").ap()
    frh_in = nc.dram_tensor("fold_rhs", [KF, MT * N], bf16, kind="ExternalInput").ap()
    flh_in = nc.dram_tensor("fold_lhsT", [KF, 128], bf16, kind="ExternalInput").ap()
    logits_in = nc.dram_tensor("logits", [RPC, C], bf16, kind="ExternalInput").ap()
    out_dram = nc.dram_tensor("out", [128, 16], f32, kind="ExternalOutput").ap()

    with tile.TileContext(nc) as tc, ExitStack() as ctx:
        resident = ctx.enter_context(tc.tile_pool(name="resident", bufs=1))
        psum_pool = ctx.enter_context(tc.tile_pool(name="psum", bufs=8, space="PSUM"))
        xent_pool = ctx.enter_context(tc.tile_pool(name="xent", bufs=2))
        small_pool = ctx.enter_context(tc.tile_pool(name="small", bufs=4))

        NP = KT // 2   # chunk pairs = 8
        ftp = [resident.tile([128, 2, N], gdt, tag=f"ftp{j}", name=f"ftp{j}")
               for j in range(NP)]
        lhp = [resident.tile([128, 2, RPC], gdt, tag=f"lhp{j}", name=f"lhp{j}")
               for j in range(NP)]
        frh = resident.tile([KF, MT, N], bf16)
        flh = resident.tile([KF, 128], bf16)
        out_tile = resident.tile([128, 16], f32)
        lg = [resident.tile([128, C], bf16, tag=f"lg{r}", name=f"lg{r}") for r in range(MT)]
        mx = [resident.tile([128, NB], f32, tag=f"mx{m}", name=f"mx{m}") for m in range(MT)]
        mn = [resident.tile([128, NB], f32, tag=f"mn{m}", name=f"mn{m}") for m in range(MT)]

        # --- input DMAs; per-pair tiles so matmul j only waits on pair j ---
        def load_pair(j):
            for i in (0, 1):
                nc.sync.dma_start(lhp[j][:, i, :], lhq_in[bass.ts(2 * j + i, 128), :])
                nc.sync.dma_start(ftp[j][:, i, :], fTq_in[bass.ts(2 * j + i, 128), :])

        load_pair(0)
        load_pair(1)
        nc.sync.dma_start(flh[:], flh_in[:])
        for r in range(MT):
            nc.sync.dma_start(lg[r][:], logits_in[bass.ts(r, 128), :])
        load_pair(2)
        for m in range(MT):
            nc.sync.dma_start(frh[:, m, :], frh_in[:, bass.ts(m, N)])
        for j in range(3, NP):
            load_pair(j)

        # --- Gram + fold + mining ---
        for m in range(MT):
            pss = [psum_pool.tile([128, 512], f32, tag="ps", name=f"ps{m}_{n}")
                   for n in range(NB)]
            if GRAM_MODE == "fp8":
                for j in range(NP):
                    w = lhp[j][:, :, bass.ts(m, 128)]
                    for n in range(NB):
                        nc.tensor.matmul(
                            pss[n][:], w, ftp[j][:, :, bass.ts(n, 512)],
                            start=(j == 0), stop=False, perf_mode=DR,
                        )
            else:
                for j in range(NP):
                    for i in (0, 1):
                        w = lhp[j][:, i, bass.ts(m, 128)]
                        for n in range(NB):
                            nc.tensor.matmul(
                                pss[n][:], w, ftp[j][:, i, bass.ts(n, 512)],
                                start=(j == 0 and i == 0), stop=False,
                            )
            for n in range(NB):
                nc.tensor.matmul(pss[n][:], flh[:], frh[:, m, bass.ts(n, 512)],
                                 start=False, stop=True)
            for n in range(NB):
                nc.vector.tensor_reduce(mx[m][:, n:n + 1], pss[n][:], axis=X, op=Alu.max)
                nc.vector.tensor_reduce(mn[m][:, n:n + 1], pss[n][:, bass.ts(m, 128)],
                                        axis=X, op=Alu.min)
            nc.vector.tensor_reduce(out_tile[:, m:m + 1], mx[m][:], axis=X, op=Alu.max)
            nc.vector.tensor_reduce(out_tile[:, 4 + m:5 + m], mn[m][:], axis=X, op=Alu.min)

            if m == 0:
                # xent: DVE row-max + negate, then ACT exp with accumulation
                negs = []
                for r in range(MT):
                    nc.vector.tensor_reduce(out_tile[:, 8 + r:9 + r], lg[r][:],
                                            axis=X, op=Alu.max)
                    neg = small_pool.tile([128, 1], f32, tag=f"neg{r}", name=f"neg{r}")
                    nc.vector.tensor_scalar_mul(neg[:], out_tile[:, 8 + r:9 + r], -1.0)
                    negs.append(neg)
                for r in range(MT):
                    escr = xent_pool.tile([128, C], bf16, tag="escr", name=f"escr{r}")
                    nc.scalar.activation(escr[:], lg[r][:], Act.Exp,
                                         bias=negs[r][:], scale=1.0,
                                         accum_out=out_tile[:, 12 + r:13 + r])

        nc.sync.dma_start(out_dram[:], out_tile[:])

    nc.compile()
    return nc


def _prepare(logits, feat, targets):
    logits = np.asarray(logits, dtype=np.float32)
    feat = np.asarray(feat, dtype=np.float32)
    targets = np.asarray(targets)

    perm = np.argsort(targets, kind="stable")
    t = np.asarray(targets)[perm]
    tg = t.reshape(-1, 4)
    assert (tg == tg[:, :1]).all(), "expected PK sampling with 4 instances/identity"

    feat_p = feat[perm]
    logits_p = logits[perm]

    gdt = ml_dtypes.float8_e4m3 if GRAM_MODE == "fp8" else ml_dtypes.bfloat16
    fq_small = feat_p.astype(gdt)                       # quantized [N, D]
    fq = fq_small.astype(np.float64)
    fTq = np.ascontiguousarray(fq_small.T)              # [D, N]
    sq = np.einsum("ij,ij->i", fq, fq).astype(np.float32)

    hi = sq.astype(ml_dtypes.bfloat16)
    lo = (sq.astype(np.float64) - hi.astype(np.float64)).astype(ml_dtypes.bfloat16)
    row_hi = (-0.5 * hi.astype(np.float32)).astype(ml_dtypes.bfloat16)
    row_lo = (-0.5 * lo.astype(np.float32)).astype(ml_dtypes.bfloat16)

    # fold lhsT [KF, 128]: rows 0,1 ones; row 2+g has 256 at cols 4g..4g+3
    flh = np.zeros((KF, 128), dtype=ml_dtypes.bfloat16)
    flh[0] = 1.0
    flh[1] = 1.0
    cols = np.arange(128)
    flh[2 + cols // 4, cols] = MASK_SCALE

    lgq = logits_p.astype(ml_dtypes.bfloat16)

    # target logit (host gather, matching jax clamp semantics)
    ti = t.astype(np.int64)
    ti = np.where(ti < 0, ti + C, ti)
    ti = np.clip(ti, 0, C - 1)
    tlog = logits_p[np.arange(N), ti].astype(np.float64)

    in_maps = []
    for c in range(NCORES):
        rows = slice(c * RPC, (c + 1) * RPC)
        frh = np.zeros((KF, MT * N), dtype=ml_dtypes.bfloat16)
        fr3 = frh.reshape(KF, MT, N)
        fr3[0, :, :] = row_hi[None, :]
        fr3[1, :, :] = row_lo[None, :]
        for m in range(MT):
            base = c * RPC + m * 128
            for g in range(32):
                fr3[2 + g, m, base + 4 * g: base + 4 * g + 4] = -MASK_SCALE

        in_maps.append({
            "fTq": fTq,
            "lhq": np.ascontiguousarray(fTq[:, rows]),
            "fold_rhs": frh,
            "fold_lhsT": flh,
            "logits": np.ascontiguousarray(lgq[rows]),
        })
    return in_maps, sq, tlog


def _combine(results, sq, tlog):
    outs = np.stack([r["out"].astype(np.float64) for r in results])  # [8, 128, 16]
    # global row (c, m, p) -> c*512 + m*128 + p
    mx = outs[:, :, 0:4].transpose(0, 2, 1).reshape(N)       # row max psum
    mn = outs[:, :, 4:8].transpose(0, 2, 1).reshape(N)       # row min window
    lmx = outs[:, :, 8:12].transpose(0, 2, 1).reshape(N)     # logits max
    les = outs[:, :, 12:16].transpose(0, 2, 1).reshape(N)    # exp sums

    sqd = sq.astype(np.float64)
    an2 = np.maximum(sqd - 2.0 * mx, 1e-12)
    ap2 = np.maximum(sqd - BIG - 2.0 * mn, 1e-12)
    trip = np.maximum(np.sqrt(ap2) - np.sqrt(an2) + MARGIN, 0.0)

    lse = lmx + np.log(les)
    xent = lse - tlog

    loss = ALPHA * xent.mean() + BETA * trip.mean()
    return np.float32(loss)


def kernel(logits, feat, targets):
    from concourse.bass_utils import run_bass_kernel_spmd

    if "nc" not in _compiled:
        _compiled["nc"] = _build_nc()
    nc = _compiled["nc"]

    in_maps, sq, tlog = _prepare(logits, feat, targets)
    res = run_bass_kernel_spmd(nc, in_maps, core_ids=list(range(NCORES)))
    return _combine(res.results, sq, tlog)
